# revision 6
# baseline (speedup 1.0000x reference)

"""Causal attention (no head split) on 8 trn2 NeuronCores.

Reference computation (per batch b):
    q = x @ Wq^T ; k = x @ Wk^T ; v = x @ Wv^T          (nn.Linear convention)
    wei = softmax(mask(q @ k^T / sqrt(C)))               (causal)
    out = wei @ v

Algebraic restructuring (K and V are never materialized):
    S   = q k^T = x (Wq^T Wk) x^T = x M x^T     with M precomputed on host
    out = wei v = (wei x) Wv^T, i.e. O^T = Wv (x^T wei^T) = Wv H
so the device only computes:
    G^T = M^T xq^T                  (one projection of this core's queries)
    S^T[s,t] = x^T(lhsT) G^T(rhs)   (contract over C)
    P^T = exp(S^T / 32) * mask ; rowsum[t] += ones^T P^T
    H[c,t] += x(lhsT) P^T(rhs)      (contract over s, accumulated in SBUF)
    O^T = Wv^T-projection of H      (once per finished query strip)
Final softmax normalization (divide by rowsum) happens on the host.

Sharding: 2 cores per batch (B=4). Queries split into eight 256-row strips;
role A takes strips {0,2,4,6} (rows [512j,512j+256)), role B {1,3,5,7}.
Every core runs the IDENTICAL instruction stream (single SPMD NEFF); role
differences are carried entirely by input data (query columns + mask tiles).

All streams are bf16 (keeps FWL enabled on the PE -> hidden weight loads,
and halves DMA). Accumulation (PSUM, H, rowsum) stays fp32.

Keys are processed in PAIRS of 256-chunks (512 keys per visit): H partial
sums accumulate across the whole pair inside PSUM (start/stop over 4
ss-subtiles), halving the vector tensor_add traffic into H, the rowsum
updates, and the number of stream DMAs. The causal structure is pair-
aligned: strip j attends exactly pairs 0..j, and the diagonal pair's mask
is the same [P, 4, SW] tile the 2-chunk scheme used.

The G phase (4 strips x 64 matmuls) is DMA-latency-bound at kernel start,
so attention work for pair 0 is interleaved between G(strips) as soon as
its gT columns exist, and DMA issue cost is spread over three otherwise-
idle engine queues (sync: weights + xt, gpsimd: xq/xna + output drain,
vector: pair-0 streams).
"""
import os
import numpy as np
import ml_dtypes

import concourse.bass as bass
from concourse import bacc
import concourse.mybir as mybir
from concourse.tile import TileContext
from concourse import bass_utils

B, T, C = 4, 2048, 1024
P = 128
CS = C // P          # 8 contraction subtiles
NPAIR = T // 512     # 4 kv chunk-pairs of 512
QS = 4               # query strips per core
SW = 256             # strip width
SCALE = 1.0 / np.sqrt(C)  # 1/32

BF16 = mybir.dt.bfloat16
F32 = mybir.dt.float32
NPBF16 = ml_dtypes.bfloat16


def build():
    nc = bacc.Bacc(trn_type="TRN2", name="causal_attn")
    xT = nc.dram_tensor("xT", [C, T], BF16, kind="ExternalInput")    # x^T (batch)
    xn = nc.dram_tensor("xn", [T, C], BF16, kind="ExternalInput")    # x natural
    xqT = nc.dram_tensor("xqT", [C, QS * SW], BF16, kind="ExternalInput")
    wm = nc.dram_tensor("wm", [C, C], BF16, kind="ExternalInput")    # M = Wq^T Wk
    wvT = nc.dram_tensor("wvT", [C, C], BF16, kind="ExternalInput")  # Wv^T [c,d]
    masks = nc.dram_tensor("masks", [P, 4, SW], BF16, kind="ExternalInput")
    ones = nc.dram_tensor("ones", [P, 1], BF16, kind="ExternalInput")
    outT = nc.dram_tensor("outT", [C, QS * SW], BF16, kind="ExternalOutput")
    rows = nc.dram_tensor("rows", [1, QS * SW], F32, kind="ExternalOutput")

    xT_r = xT.rearrange("(cs p) t -> p cs t", p=P)
    xn_r4 = xn.rearrange("(pp ss p) c -> p pp ss c", p=P, ss=4)
    xqT_r = xqT.rearrange("(cs p) t -> p cs t", p=P)
    wm_r = wm.rearrange("(cs p) d -> p cs d", p=P)
    wvT_r = wvT.rearrange("(cs p) d -> p cs d", p=P)
    outT_r = outT.rearrange("(ds p) t -> p ds t", p=P)
    rows_r = rows.rearrange("p (a b) -> p a b", a=QS)

    with TileContext(nc) as tc:
        with tc.tile_pool(name="keep", bufs=1) as keep, \
             tc.tile_pool(name="wpool", bufs=2) as wpool, \
             tc.tile_pool(name="qpool", bufs=4) as qpool, \
             tc.tile_pool(name="stream", bufs=3) as stream, \
             tc.tile_pool(name="hrpool", bufs=2) as hrpool, \
             tc.tile_pool(name="ppool", bufs=3) as ppool, \
             tc.tile_pool(name="psA", bufs=2, space="PSUM") as psA, \
             tc.tile_pool(name="psS", bufs=3, space="PSUM") as psS, \
             tc.tile_pool(name="psO", bufs=2, space="PSUM") as psO, \
             tc.tile_pool(name="psR", bufs=1, space="PSUM") as psR:

            gT = keep.tile([P, CS, QS * SW], BF16, tag="gT")   # G^T  16KB/part
            hh = keep.tile([P, CS, QS * SW], F32, tag="hh")    # H    32KB/part
            msk = keep.tile([P, 4, SW], BF16, tag="msk")
            ones_t = keep.tile([P, 1], BF16, tag="ones")
            rowsum = keep.tile([1, QS, SW], F32, tag="rowsum")

            # ---- prologue DMA: first matmul needs wq[:, 0:2, 0:P] + xq0 cs0.
            # sync carries wm/wv/xt, gpsimd carries xq/masks/xna, vector the
            # pair-0 streams; fine-grained first pieces, coarse afterwards.
            wq = wpool.tile([P, CS, C], BF16, tag="w")
            nc.sync.dma_start(wq[:, 0:2, 0:P], wm_r[:, 0:2, 0:P])
            nc.sync.dma_start(wq[:, 2:8, 0:P], wm_r[:, 2:8, 0:P])
            nc.sync.dma_start(wq[:, :, P:2 * P], wm_r[:, :, P:2 * P])
            nc.sync.dma_start(wq[:, :, 2 * P:4 * P], wm_r[:, :, 2 * P:4 * P])
            nc.sync.dma_start(wq[:, :, 4 * P:6 * P], wm_r[:, :, 4 * P:6 * P])
            nc.sync.dma_start(wq[:, :, 6 * P:8 * P], wm_r[:, :, 6 * P:8 * P])
            wv = wpool.tile([P, CS, C], BF16, tag="w")
            nc.sync.dma_start(wv[:, :, 0:512], wvT_r[:, :, 0:512])
            nc.sync.dma_start(wv[:, :, 512:1024], wvT_r[:, :, 512:1024])

            xq = []
            for j in range(QS):
                xq.append(qpool.tile([P, CS, SW], BF16, tag="xq",
                                     name=f"xq{j}"))
            jsl = lambda j: slice(j * SW, (j + 1) * SW)
            nc.gpsimd.dma_start(xq[0][:, 0:4], xqT_r[:, 0:4, 0:SW])
            nc.gpsimd.dma_start(xq[0][:, 4:8], xqT_r[:, 4:8, 0:SW])
            nc.gpsimd.dma_start(ones_t[:], ones[:])
            nc.gpsimd.dma_start(msk[:], masks[:])
            for j in range(1, QS):
                nc.gpsimd.dma_start(xq[j][:], xqT_r[:, :, jsl(j)])

            def stream_pair(p, engine):
                xt2 = stream.tile([P, CS, 512], BF16, tag="xt")
                ksl = slice(p * 512, (p + 1) * 512)
                engine.dma_start(xt2[:, 0:4], xT_r[:, 0:4, ksl])
                engine.dma_start(xt2[:, 4:8], xT_r[:, 4:8, ksl])
                xna2 = stream.tile([P, 4, C], BF16, tag="xn")
                nc.gpsimd.dma_start(xna2[:], xn_r4[:, p])
                return xt2, xna2

            pair0 = stream_pair(0, nc.scalar)

            def G(j):
                for ds in range(CS):
                    pq = psA.tile([P, SW], F32, tag="prod")
                    for cs in range(CS):
                        nc.tensor.matmul(
                            pq[:], wq[:, cs, ds * P:(ds + 1) * P], xq[j][:, cs],
                            start=(cs == 0), stop=(cs == CS - 1))
                    nc.scalar.copy(gT[:, ds, jsl(j)], pq[:])

            def SH(p, j, xt2, xna2):
                tsl = jsl(j)
                pT = ppool.tile([P, 4, SW], BF16, tag="pT")
                for half in range(2):
                    st = psS.tile([P, 2, SW], F32, tag="st")
                    for sl in range(2):
                        ss = 2 * half + sl
                        for cs in range(CS):
                            nc.tensor.matmul(
                                st[:, sl], xt2[:, cs, ss * P:(ss + 1) * P],
                                gT[:, cs, tsl],
                                start=(cs == 0), stop=(cs == CS - 1))
                    nc.scalar.activation(
                        pT[:, 2 * half:2 * half + 2], st[:],
                        mybir.ActivationFunctionType.Exp, scale=float(SCALE))
                if p == j:     # diagonal pair: causal mask via data tile
                    nc.vector.tensor_mul(pT[:], pT[:], msk[:])

                rw = psR.tile([1, SW], F32, tag="rw")
                for ss in range(4):
                    nc.tensor.matmul(
                        rw[:], ones_t[:], pT[:, ss],
                        start=(ss == 0), stop=(ss == 3))
                if p == 0:
                    nc.vector.tensor_copy(rowsum[:, j], rw[:])
                else:
                    nc.vector.tensor_add(rowsum[:, j], rowsum[:, j], rw[:])

                # H[c,t] += x(lhsT) @ P^T; whole 512-key pair accumulates in
                # PSUM before a single vector add per c-quarter.
                for q4 in range(4):
                    po = psO.tile([P, 2, SW], F32, tag="po")
                    for i in range(2):
                        cs4 = 2 * q4 + i
                        for ss in range(4):
                            nc.tensor.matmul(
                                po[:, i], xna2[:, ss, cs4 * P:(cs4 + 1) * P],
                                pT[:, ss],
                                start=(ss == 0), stop=(ss == 3))
                    hsl = hh[:, 2 * q4:2 * q4 + 2, tsl]
                    if p == 0:
                        nc.vector.tensor_copy(hsl, po[:])
                    else:
                        nc.vector.tensor_add(hsl, hsl, po[:])

            def proj(j):
                # strip j's H is complete: O^T = Wv^T-projection, drained
                # (cast + outT DMA) on the gpsimd queue in-order.
                tsl = jsl(j)
                hr = hrpool.tile([P, CS, SW], BF16, tag="hr")
                for q2 in range(2):
                    nc.scalar.copy(hr[:, 4 * q2:4 * q2 + 4],
                                   hh[:, 4 * q2:4 * q2 + 4, tsl])
                ost = hrpool.tile([P, CS, SW], BF16, tag="ost")
                for ds in range(CS):
                    pf = psA.tile([P, SW], F32, tag="prod")
                    for cs in range(CS):
                        nc.tensor.matmul(
                            pf[:], wv[:, cs, ds * P:(ds + 1) * P], hr[:, cs],
                            start=(cs == 0), stop=(cs == CS - 1))
                    nc.vector.tensor_copy(ost[:, ds], pf[:])
                    if ds % 2 == 1:
                        nc.gpsimd.dma_start(
                            outT_r[:, ds - 1:ds + 1, tsl],
                            ost[:, ds - 1:ds + 1])

            # ---- emission: interleave G with pair-0 attention so the PE has
            # work while wq/xq stream in; pair order ends on pair 2 so strips
            # 2 and 3 complete together and their projections interleave.
            G(0)
            G(1)
            SH(0, 0, *pair0)
            SH(0, 1, *pair0)
            proj(0)
            pair1 = stream_pair(1, nc.sync)
            G(2)
            SH(0, 2, *pair0)
            G(3)
            SH(0, 3, *pair0)
            pair3 = stream_pair(3, nc.sync)
            SH(1, 1, *pair1)
            SH(1, 2, *pair1)
            proj(1)
            SH(1, 3, *pair1)
            pair2 = stream_pair(2, nc.sync)
            SH(3, 3, *pair3)
            SH(2, 2, *pair2)
            SH(2, 3, *pair2)
            proj(2)
            proj(3)

            nc.sync.dma_start(rows_r[:], rowsum[:])

    nc.compile()
    return nc


_NC = None


def _get_nc():
    global _NC
    if _NC is None:
        _NC = build()
    return _NC


def make_in_maps(x, Wq, Wk, Wv):
    x = np.asarray(x, dtype=np.float32)
    wq64 = np.asarray(Wq, np.float64)
    wk64 = np.asarray(Wk, np.float64)
    wm = (wq64.T @ wk64).astype(NPBF16)                      # M = Wq^T Wk [c',c]
    wvT = np.ascontiguousarray(np.asarray(Wv, np.float32).T).astype(NPBF16)
    ones = np.ones((P, 1), NPBF16)

    # mask tiles [p, ss, t] over a diagonal 512-key pair: visible iff
    # (key offset within pair) <= (query offset within strip) + 256*role
    s_idx = (np.arange(2)[:, None, None] * P + np.arange(P)[None, :, None])
    tri = (s_idx <= np.arange(SW)[None, None, :]).astype(np.float32)
    tri = np.ascontiguousarray(tri.transpose(1, 0, 2))
    zeros = np.zeros((P, 2, SW), np.float32)
    ones2 = np.ones((P, 2, SW), np.float32)
    mask_A = np.concatenate([tri, zeros], axis=1).astype(NPBF16)
    mask_B = np.concatenate([ones2, tri], axis=1).astype(NPBF16)

    xr = [x[b].astype(NPBF16) for b in range(B)]
    xT = [np.ascontiguousarray(xr[b].T) for b in range(B)]
    in_maps = []
    for core in range(8):
        b, role = divmod(core, 2)
        cols = np.concatenate(
            [np.arange(512 * j + SW * role, 512 * j + SW * role + SW)
             for j in range(QS)])
        xqT = np.ascontiguousarray(xT[b][:, cols])
        in_maps.append({
            "xT": xT[b],
            "xn": xr[b],
            "xqT": xqT,
            "wm": wm, "wvT": wvT,
            "masks": mask_A if role == 0 else mask_B,
            "ones": ones,
        })
    return in_maps


def assemble(results):
    out = np.empty((B, T, C), np.float32)
    for core in range(8):
        b, role = divmod(core, 2)
        oT = np.asarray(results[core]["outT"]).astype(np.float32)  # [C, 1024]
        rsum = np.asarray(results[core]["rows"]).reshape(QS * SW)
        o = oT.T / rsum[:, None]
        for j in range(QS):
            r0 = 512 * j + SW * role
            out[b, r0:r0 + SW] = o[j * SW:(j + 1) * SW]
    return out


def kernel(x, Wq, Wk, Wv):
    nc = _get_nc()
    in_maps = make_in_maps(x, Wq, Wk, Wv)
    res = bass_utils.run_bass_kernel_spmd(nc, in_maps, core_ids=list(range(8)))
    return assemble(res.results)


def _install_trace_shim():
    """Provide antenv.axon_hooks (absent in this image) so trace=True works."""
    import sys
    import types
    if "antenv.axon_hooks" in sys.modules:
        return
    hook_box = [None]
    mod = types.ModuleType("antenv.axon_hooks")
    mod.set_axon_ntff_profile_hook = lambda h: hook_box.__setitem__(0, h)
    mod.get_axon_ntff_profile_hook = lambda: hook_box[0]
    import antenv
    sys.modules["antenv.axon_hooks"] = mod
    antenv.axon_hooks = mod
    try:
        from trn_agent_boot.trn_boot import _ntff_profile_via_ctypes
        mod.set_axon_ntff_profile_hook(
            _ntff_profile_via_ctypes("/opt/axon/libaxon_pjrt.so"))
    except Exception:
        pass


def run_traced(x, Wq, Wk, Wv):
    """Like kernel() but with NTFF tracing; returns (out, BassKernelResults)."""
    _install_trace_shim()
    nc = _get_nc()
    in_maps = make_in_maps(x, Wq, Wk, Wv)
    res = bass_utils.run_bass_kernel_spmd(
        nc, in_maps, core_ids=list(range(8)), trace=True,
        trace_cores=list(range(8)))
    return assemble(res.results), res
